# revision 1
# baseline (speedup 1.0000x reference)
"""Competing-risk TabM loss (Cox PH partial likelihood + cross-entropy) on
8 Trainium2 NeuronCores.

Strategy (data-parallel over N, one bass launch):
  host:   stable argsort of -durations; permute log_h/event_type into sorted
          order; pad each core's segment of 125000 rows to 128*980 and
          reshape to [128 partitions, 980, ...]; logits/labels stay in
          natural order (CE is permutation-invariant).
  device: per core, stream the log_h shard, reduce over the M=8 ensemble
          heads, w = exp(eta), per-partition inclusive cumsum via
          tensor_tensor_scan, partition-level exclusive prefix via a
          triangular-ones matmul, cross-core prefix via a 16-byte AllGather
          plus a per-core selection matmul, then log(denom+eps) fused into
          one activation (per-partition bias), masked per-cause reductions
          via scalar_tensor_tensor with fused accum_out.  The CE shard is
          streamed independently (mean over heads, logsumexp, one-hot pick).
  host:   sum the [128, ...] per-core partials in float64 and assemble the
          scalar loss.
"""

import os
from contextlib import ExitStack

import numpy as np

os.environ.setdefault("JAX_PLATFORMS", "axon")

from concourse import bacc, mybir
import concourse.tile as tile
from concourse.bass_utils import run_bass_kernel_spmd

# problem constants (hardcoded per task spec)
N = 1_000_000
M = 8
K = 4
NUM_CLS = K + 1
ALPHA = 0.4
EPS = 1e-8

P = 128
N_CORES = 8

F32 = mybir.dt.float32
X = mybir.AxisListType = mybir.AxisListType
ADD = mybir.AluOpType.add
MULT = mybir.AluOpType.mult
ISEQ = mybir.AluOpType.is_equal
ISGE = mybir.AluOpType.is_ge
EXP = mybir.ActivationFunctionType.Exp
LN = mybir.ActivationFunctionType.Ln
COPY = mybir.ActivationFunctionType.Copy


def build_nc(L, tc_lh, tc_lg, reps=1, loop_reps=1, single_core=False):
    """Build the per-core bass program.  L = padded rows per partition.

    reps > 1 re-executes the whole body serially inside one NEFF; used to
    measure steady-state per-iteration device time without dispatch
    overhead (outputs are simply overwritten each rep)."""
    assert L % tc_lh == 0 and L % tc_lg == 0
    n_lh = L // tc_lh
    n_lg = L // tc_lg

    nc = bacc.Bacc("TRN2", debug=False,
                   num_devices=1 if single_core else N_CORES)
    lh = nc.dram_tensor("lh", [P, L * M * K], F32, kind="ExternalInput")
    lg = nc.dram_tensor("lg", [P, L * M * NUM_CLS], F32, kind="ExternalInput")
    ev = nc.dram_tensor("ev", [P, L], F32, kind="ExternalInput")
    lb = nc.dram_tensor("lb", [P, L], F32, kind="ExternalInput")
    tri = nc.dram_tensor("tri", [P, P], F32, kind="ExternalInput")
    sel = nc.dram_tensor("sel", [N_CORES, P], F32, kind="ExternalInput")
    accs = nc.dram_tensor("accs", [P, 20], F32, kind="ExternalOutput")
    # collectives cannot touch I/O tensors -> internal DRAM bounce buffers
    cc_in = nc.dram_tensor("cc_in", [1, K], F32)
    cc_out = nc.dram_tensor("cc_out", [N_CORES, K], F32)

    with tile.TileContext(nc) as tc, ExitStack() as ctx:
        persist = ctx.enter_context(tc.tile_pool(name="persist", bufs=1))
        lhp = ctx.enter_context(tc.tile_pool(name="lhp", bufs=3))
        lgp = ctx.enter_context(tc.tile_pool(name="lgp", bufs=3))
        work = ctx.enter_context(tc.tile_pool(name="work", bufs=2))
        psum = ctx.enter_context(tc.tile_pool(name="psum", bufs=1, space="PSUM"))

        def emit_rep():

            wbuf = persist.tile([P, L, K], F32)     # exp(eta), row-major
            cumbuf = persist.tile([P, L, K], F32)   # per-partition inclusive cumsum
            evt = persist.tile([P, L], F32)
            lbt = persist.tile([P, L], F32)
            trit = persist.tile([P, P], F32)
            selt = persist.tile([N_CORES, P], F32)
            ones = persist.tile([P, 1], F32)
            zerosL = persist.tile([P, L], F32)
            onesL = persist.tile([P, L], F32)
            totals = persist.tile([P, K], F32)      # per-partition scan totals
            g8 = persist.tile([N_CORES, K], F32)    # gathered per-core totals
            comb = persist.tile([P, K], F32)        # partition-prefix + EPS
            combf = persist.tile([P, K], F32)       # + core prefix
            eta_s = persist.tile([P, K * n_lh], F32)
            nev_s = persist.tile([P, K * n_lh], F32)
            lse_s = persist.tile([P, n_lg], F32)
            pick_s = persist.tile([P, NUM_CLS * n_lg], F32)
            logd_s = persist.tile([P, K], F32)
            acc_out = persist.tile([P, 20], F32)

            nc.sync.dma_start(evt[:], ev[:, :])
            nc.sync.dma_start(lbt[:], lb[:, :])
            nc.sync.dma_start(trit[:], tri[:, :])
            nc.sync.dma_start(selt[:], sel[:, :])
            nc.vector.memset(ones[:], 1.0)
            nc.vector.memset(zerosL[:], 0.0)
            nc.vector.memset(onesL[:], 1.0)

            # ---------- Cox stream: sorted log_h ----------
            for i in range(n_lh):
                t = lhp.tile([P, tc_lh * M * K], F32, tag="lht")
                nc.sync.dma_start(t[:], lh[:, i * tc_lh * M * K:(i + 1) * tc_lh * M * K])
                # memory per row: (t, m, k); reduce heads via contiguous
                # tree adds (full DVE rate; strided X-reduce is ~4x slower)
                tv = t[:].rearrange("p (t x) -> p t x", t=tc_lh, x=M * K)
                # in-place halving tree (saves SBUF; elementwise out==in0 is safe)
                nc.vector.tensor_add(tv[:, :, 0:M * K // 2], tv[:, :, 0:M * K // 2],
                                     tv[:, :, M * K // 2:M * K])
                nc.vector.tensor_add(tv[:, :, 0:M * K // 4], tv[:, :, 0:M * K // 4],
                                     tv[:, :, M * K // 4:M * K // 2])
                ms = work.tile([P, tc_lh, K], F32, tag="msum")   # head-sum, row-major
                nc.vector.tensor_add(ms[:], tv[:, :, 0:K], tv[:, :, K:2 * K])
                nc.scalar.activation(wbuf[:, i * tc_lh:(i + 1) * tc_lh, :], ms[:],
                                     EXP, bias=0.0, scale=1.0 / M)
                evs = evt[:, i * tc_lh:(i + 1) * tc_lh]
                scr = work.tile([P, tc_lh], F32, tag="scr")
                for k in range(K):
                    c = i * K + k
                    # masked eta sum (in units of the head-sum; /M on host)
                    nc.vector.scalar_tensor_tensor(
                        scr[:], evs, float(k + 1), ms[:, :, k], ISEQ, MULT,
                        accum_out=eta_s[:, c:c + 1])
                    # event count
                    nc.vector.scalar_tensor_tensor(
                        scr[:], evs, float(k + 1),
                        onesL[:, i * tc_lh:(i + 1) * tc_lh], ISEQ, MULT,
                        accum_out=nev_s[:, c:c + 1])

            # ---------- cumulative risk-set denominators ----------
            for k in range(K):
                nc.vector.tensor_tensor_scan(
                    cumbuf[:, :, k], zerosL[:], wbuf[:, :, k], 0.0,
                    op0=ADD, op1=ADD)
                nc.vector.tensor_copy(totals[:, k:k + 1], cumbuf[:, L - 1, k:k + 1])

            # partition-level exclusive prefix: tri[q,p] = 1 iff q < p
            pa = psum.tile([P, K], F32, tag="pa")
            nc.tensor.matmul(pa[:], trit[:], totals[:], start=True, stop=True)
            # whole-core totals, broadcast to partition 0
            pc = psum.tile([P, K], F32, tag="pc")
            nc.tensor.matmul(pc[:1, :], ones[:], totals[:], start=True, stop=True)
            ct = persist.tile([1, K], F32)
            nc.scalar.copy(ct[:], pc[0:1, :])
            nc.sync.dma_start(cc_in[0:1, :], ct[:])
            if single_core:
                # timeline-sim variant: collectives unsupported; substitute a
                # same-shape DRAM round-trip (timing-only stand-in)
                g8m = persist.tile([N_CORES, K], F32)
                nc.vector.memset(g8m[:], 0.5)
                nc.sync.dma_start(cc_out[:, :], g8m[:])
            else:
                nc.gpsimd.collective_compute(
                    "AllGather", mybir.AluOpType.bypass,
                    replica_groups=[list(range(N_CORES))],
                    ins=[cc_in[:, :]], outs=[cc_out[:, :]],
                )
            nc.sync.dma_start(g8[:], cc_out[:, :])
            # cross-core exclusive prefix, broadcast across partitions:
            # sel[q, p] = 1 iff q < core_id  (per-core constant input)
            pb = psum.tile([P, K], F32, tag="pb")
            nc.tensor.matmul(pb[:], selt[:], g8[:], start=True, stop=True)
            nc.scalar.activation(comb[:], pa[:], COPY, bias=EPS)
            nc.vector.tensor_add(combf[:], comb[:], pb[:])

            # log(denom + eps) and masked per-cause sums
            for k in range(K):
                logd = work.tile([P, L], F32, tag="logd")
                nc.scalar.activation(logd[:], cumbuf[:, :, k], LN,
                                     bias=combf[:, k:k + 1], scale=1.0)
                scrL = work.tile([P, L], F32, tag="scrL")
                nc.vector.scalar_tensor_tensor(
                    scrL[:], evt[:], float(k + 1), logd[:], ISEQ, MULT,
                    accum_out=logd_s[:, k:k + 1])

            # ---------- CE stream: natural-order logits ----------
            for i in range(n_lg):
                t = lgp.tile([P, tc_lg * M * NUM_CLS], F32, tag="lgt")
                nc.sync.dma_start(
                    t[:], lg[:, i * tc_lg * M * NUM_CLS:(i + 1) * tc_lg * M * NUM_CLS])
                tv = t[:].rearrange("p (t x) -> p t x", t=tc_lg, x=M * NUM_CLS)
                nc.vector.tensor_add(tv[:, :, 0:M * NUM_CLS // 2],
                                     tv[:, :, 0:M * NUM_CLS // 2],
                                     tv[:, :, M * NUM_CLS // 2:M * NUM_CLS])
                nc.vector.tensor_add(tv[:, :, 0:M * NUM_CLS // 4],
                                     tv[:, :, 0:M * NUM_CLS // 4],
                                     tv[:, :, M * NUM_CLS // 4:M * NUM_CLS // 2])
                ls = work.tile([P, tc_lg, NUM_CLS], F32, tag="lsum")
                nc.vector.tensor_add(ls[:], tv[:, :, 0:NUM_CLS],
                                     tv[:, :, NUM_CLS:2 * NUM_CLS])
                e = work.tile([P, tc_lg, NUM_CLS], F32, tag="e")
                nc.scalar.activation(e[:], ls[:], EXP, bias=0.0, scale=1.0 / M)
                se = work.tile([P, tc_lg], F32, tag="se")
                nc.vector.tensor_reduce(se[:], e[:], axis=X.X, op=ADD)
                lse = work.tile([P, tc_lg], F32, tag="lse")
                nc.scalar.activation(lse[:], se[:], LN, bias=0.0, scale=1.0)
                lbs = lbt[:, i * tc_lg:(i + 1) * tc_lg]
                scr = work.tile([P, tc_lg], F32, tag="scrce")
                # rows are padded with label = -1 -> excluded via is_ge mask
                nc.vector.scalar_tensor_tensor(
                    scr[:], lbs, 0.0, lse[:], ISGE, MULT,
                    accum_out=lse_s[:, i:i + 1])
                for j in range(NUM_CLS):
                    nc.vector.scalar_tensor_tensor(
                        scr[:], lbs, float(j), ls[:, :, j], ISEQ, MULT,
                        accum_out=pick_s[:, i * NUM_CLS + j:i * NUM_CLS + j + 1])

            # ---------- final reductions -> accs ----------
            ev_eta = eta_s[:].rearrange("p (i k) -> p k i", i=n_lh, k=K)
            nc.vector.tensor_reduce(acc_out[:, 0:K], ev_eta, axis=X.X, op=ADD)
            ev_nev = nev_s[:].rearrange("p (i k) -> p k i", i=n_lh, k=K)
            nc.vector.tensor_reduce(acc_out[:, K:2 * K], ev_nev, axis=X.X, op=ADD)
            nc.vector.tensor_copy(acc_out[:, 2 * K:3 * K], logd_s[:])
            nc.vector.tensor_reduce(acc_out[:, 12:13], lse_s[:], axis=X.X, op=ADD)
            nc.vector.tensor_reduce(acc_out[:, 13:14], pick_s[:], axis=X.X, op=ADD)
            nc.vector.tensor_copy(acc_out[:, 14:18], totals[:])
            nc.vector.memset(acc_out[:, 18:20], 0.0)
            nc.sync.dma_start(accs[:, :], acc_out[:])


        # reps/loop_reps > 1 repeat the body for steady-state timing
        if loop_reps > 1:
            with tc.For_i(0, loop_reps, 1):
                emit_rep()
        else:
            for _rep in range(reps):
                emit_rep()

    nc.finalize()
    return nc


def prep_inputs(log_h, logits, durations, event_type, labels, L):
    """Host-side shard/permute.  Returns per-core in_maps."""
    n = log_h.shape[0]
    per_core = n // N_CORES
    assert per_core * N_CORES == n
    pl = P * L
    pad = pl - per_core
    assert pad >= 0

    order = np.argsort(-durations, kind="stable")
    lh_s = np.ascontiguousarray(log_h[order]).reshape(n, M * K)
    ev_s = event_type[order].astype(np.float32)

    lg_n = np.ascontiguousarray(logits).reshape(n, M * NUM_CLS)
    lb_n = labels.astype(np.float32)

    tri = np.triu(np.ones((P, P), np.float32), 1)  # tri[q,p]=1 iff q<p

    in_maps = []
    for c in range(N_CORES):
        s = slice(c * per_core, (c + 1) * per_core)
        lh_c = np.full((pl, M * K), -1e9, np.float32)
        lh_c[:per_core] = lh_s[s]
        ev_c = np.zeros(pl, np.float32)
        ev_c[:per_core] = ev_s[s]
        lg_c = np.zeros((pl, M * NUM_CLS), np.float32)
        lg_c[:per_core] = lg_n[s]
        lb_c = np.full(pl, -1.0, np.float32)
        lb_c[:per_core] = lb_n[s]
        sel = np.zeros((N_CORES, P), np.float32)
        sel[:c, :] = 1.0
        in_maps.append({
            "lh": lh_c.reshape(P, L * M * K),
            "lg": lg_c.reshape(P, L * M * NUM_CLS),
            "ev": ev_c.reshape(P, L),
            "lb": lb_c.reshape(P, L),
            "tri": tri,
            "sel": sel,
        })
    return in_maps


def combine(results, n):
    """Host-side f64 combine of the per-core [128,20] partials."""
    a = np.stack([r["accs"] for r in results]).astype(np.float64)  # [C,P,20]
    s = a.sum(axis=(0, 1))  # [20]
    s_eta = s[0:K] / M
    n_ev = s[K:2 * K]
    s_logd = s[2 * K:3 * K]
    s_lse = s[12]
    s_pick = s[13] / M
    loss_c = -(s_eta - s_logd) / (n_ev + EPS)
    loss_surv = loss_c.sum()
    loss_cls = (s_lse - s_pick) / n
    return np.float32(ALPHA * loss_surv + (1.0 - ALPHA) * loss_cls)


_NC_CACHE = {}


def _get_nc(L, tc_lh, tc_lg, reps=1, loop_reps=1):
    key = (L, tc_lh, tc_lg, reps, loop_reps)
    if key not in _NC_CACHE:
        _NC_CACHE[key] = build_nc(L, tc_lh, tc_lg, reps, loop_reps)
    return _NC_CACHE[key]


def run(log_h, logits, durations, event_type, labels, L, tc_lh, tc_lg):
    nc = _get_nc(L, tc_lh, tc_lg)
    in_maps = prep_inputs(log_h, logits, durations, event_type, labels, L)
    try:
        res = run_bass_kernel_spmd(nc, in_maps, list(range(N_CORES)))
    except Exception as e:  # transient NRT_EXEC_UNIT_UNRECOVERABLE after fresh compile
        if "UNRECOVERABLE" not in str(e) and "UNAVAILABLE" not in str(e):
            raise
        res = run_bass_kernel_spmd(nc, in_maps, list(range(N_CORES)))
    return combine(res.results, log_h.shape[0])


def _make_runner(nc, in_maps):
    """Steady-state runner: jitted shard_map with device-resident inputs.

    Returns a zero-arg callable executing one kernel launch (blocking)."""
    import jax
    from jax.sharding import Mesh, PartitionSpec, NamedSharding
    from jax.experimental.shard_map import shard_map
    from concourse import bass2jax, mybir as mb

    bass2jax.install_neuronx_cc_hook()
    in_names, out_names, out_avals, zero_outs = [], [], [], []
    partition_name = nc.partition_id_tensor.name if nc.partition_id_tensor else None
    for alloc in nc.m.functions[0].allocations:
        if not isinstance(alloc, mb.MemoryLocationSet):
            continue
        name = alloc.memorylocations[0].name
        if alloc.kind == "ExternalInput":
            if name != partition_name:
                in_names.append(name)
        elif alloc.kind == "ExternalOutput":
            out_names.append(name)
            out_avals.append(jax.core.ShapedArray(
                tuple(alloc.tensor_shape), mb.dt.np(alloc.dtype)))
            zero_outs.append(np.zeros(alloc.tensor_shape, mb.dt.np(alloc.dtype)))
    n_params = len(in_names)
    n_outs = len(out_names)
    all_in_names = list(in_names) + list(out_names)
    if partition_name is not None:
        all_in_names.append(partition_name)

    def _body(*args):
        operands = list(args)
        if partition_name is not None:
            operands.append(bass2jax.partition_id_tensor())
        outs = bass2jax._bass_exec_p.bind(
            *operands,
            out_avals=tuple(out_avals),
            in_names=tuple(all_in_names),
            out_names=tuple(out_names),
            lowering_input_output_aliases=(),
            sim_require_finite=True,
            sim_require_nnan=True,
            nc=nc,
        )
        return tuple(outs)

    devices = jax.devices()[:N_CORES]
    mesh = Mesh(np.asarray(devices), ("core",))
    in_specs = (PartitionSpec("core"),) * (n_params + n_outs)
    out_specs = (PartitionSpec("core"),) * n_outs
    sharded = jax.jit(
        shard_map(_body, mesh=mesh, in_specs=in_specs, out_specs=out_specs,
                  check_rep=False),
        donate_argnums=tuple(range(n_params, n_params + n_outs)),
        keep_unused=True,
    )
    sh = NamedSharding(mesh, PartitionSpec("core"))
    dev_in = [
        jax.device_put(
            np.concatenate([np.asarray(in_maps[c][nm]) for c in range(N_CORES)],
                           axis=0), sh)
        for nm in in_names
    ]

    def call():
        zeros = [np.zeros((N_CORES * z.shape[0], *z.shape[1:]), z.dtype)
                 for z in zero_outs]
        outs = sharded(*dev_in, *zeros)
        jax.block_until_ready(outs)
        return outs

    return call


def measure_exec_ns(inputs, L=980, tc_lh=140, tc_lg=98, iters=8):
    """Median steady-state wall time of one launch with device-resident
    inputs, minus the same measurement for a trivial null program (dispatch
    floor).  Best available proxy for HW exec time (no NTFF profiling hook
    in this container)."""
    import time

    nc = _get_nc(L, tc_lh, tc_lg)
    in_maps = prep_inputs(np.asarray(inputs["log_h"], np.float32),
                          np.asarray(inputs["logits"], np.float32),
                          np.asarray(inputs["durations"], np.float32),
                          np.asarray(inputs["event_type"]),
                          np.asarray(inputs["labels"]), L)
    call = _make_runner(nc, in_maps)

    def tmin(fn, n):
        ts = []
        for _ in range(n):
            t0 = time.perf_counter()
            fn()
            ts.append(time.perf_counter() - t0)
        return min(ts)

    call()  # warm
    t_kernel = tmin(call, iters)

    # slope between reps=1 and reps=31 in one NEFF cancels the ~75-100 ms
    # per-call axon dispatch overhead (no NTFF profiling in this container)
    r_hi = 31
    call_hi = _make_runner(_get_nc(L, tc_lh, tc_lg, reps=r_hi), in_maps)
    call_hi()
    t_hi = tmin(call_hi, iters)
    per_iter = (t_hi - t_kernel) / (r_hi - 1)
    print(f"  [steady-state wall: reps=1 {t_kernel*1e6:.0f} us, "
          f"reps={r_hi} {t_hi*1e6:.0f} us -> {per_iter*1e6:.1f} us/iter]")
    return max(per_iter, 0.0) * 1e9


def _get_null_nc():
    if "null" not in _NC_CACHE:
        nc = bacc.Bacc("TRN2", debug=False, num_devices=N_CORES)
        nx = nc.dram_tensor("nx", [P, 4], F32, kind="ExternalInput")
        ny = nc.dram_tensor("ny", [P, 4], F32, kind="ExternalOutput")
        with tile.TileContext(nc) as tc:
            with tc.tile_pool(name="p", bufs=1) as pool:
                t = pool.tile([P, 4], F32)
                nc.sync.dma_start(t[:], nx[:, :])
                nc.sync.dma_start(ny[:, :], t[:])
        nc.finalize()
        _NC_CACHE["null"] = nc
    return _NC_CACHE["null"]


def kernel(log_h, logits, durations, event_type, labels):
    log_h = np.asarray(log_h, dtype=np.float32)
    logits = np.asarray(logits, dtype=np.float32)
    durations = np.asarray(durations, dtype=np.float32)
    event_type = np.asarray(event_type)
    labels = np.asarray(labels)
    out = run(log_h, logits, durations, event_type, labels,
              L=980, tc_lh=140, tc_lg=98)
    return np.array(out, dtype=np.float32)



# revision 11
# speedup vs baseline: 1.9622x; 1.9622x over previous
"""Competing-risk TabM loss (Cox PH partial likelihood + cross-entropy) on
8 Trainium2 NeuronCores — fp8 / TensorEngine edition.

Strategy (data-parallel over N, one bass launch):
  host:   argsort of -durations; permute log_h/event_type into sorted order;
          quantize log_h/logits to fp8-e4m3 and pack them into a layout where
          the 8-head reduction becomes a DoubleRow fp8 matmul (256-wide
          contraction) on the PE array; event one-hots / label match masks are
          shipped as fp8-e3m4 (small ints are exact).
  device: Cox: 32 DR matmuls produce head-sums for 4 causes x 32 row-slots
          across all 128 psum partitions; exp via ACT (scale=1/M); one
          full-width tensor_tensor_scan gives per-partition inclusive
          cumsums; slot-prefix and per-cause core totals via two small fp32
          matmuls; cross-core prefix via a 512B AllGather plus one matmul;
          log-denominators in one ACT Ln with per-partition bias; masked
          reductions via DVE stt with accum_out.
          CE: 44 DR matmuls reduce heads; ACT exp -> bf16; class-sum via a
          [128x24] bf16 matmul; ACT Ln with accum_out sums the logsumexp
          directly (pad rows encode ln(1/NUM_CLS) so they contribute ~0 and
          the exact residual is corrected on the host); label-pick via stt.
  host:   f64 combine of the [128, 48] per-core partials.
"""

import os
from contextlib import ExitStack

import numpy as np
import ml_dtypes

os.environ.setdefault("JAX_PLATFORMS", "axon")

from concourse import bacc, mybir
import concourse.tile as tile
from concourse.bass_utils import run_bass_kernel_spmd

# problem constants (hardcoded per task spec)
N = 1_000_000
M = 8
K = 4
NUM_CLS = K + 1
ALPHA = 0.4
EPS = 1e-8

P = 128
N_CORES = 8
RC = N // N_CORES           # 125000 real rows per core

# Cox layout: 32 slots x 4096 rows, 32 DR matmuls of 512 col-pairs
LH_ROWS = 131072            # padded rows per core
LH_SLOT = 4096              # rows per (partition) slot
LH_MM = 32                  # DR matmuls (each covers 8 rows x 512 columns)
LH_G = 8                    # groups of 4 matmuls -> one [128, 512] psum bank

# CE layout: 44 DR matmuls of 6 rows x 512 columns, 11 groups of 4
LG_MM = 44
LG_G = 11
LG_ROWS = LG_MM * 6 * 512   # 135168 padded rows per core
PADCE = float(np.log(1.0 / NUM_CLS))  # pad logit value -> lse contribution ~0

F32 = mybir.dt.float32
BF16 = mybir.dt.bfloat16
FP8E4 = mybir.dt.float8e4
FP8E3 = mybir.dt.float8e3
E4NP = ml_dtypes.float8_e4m3
E3NP = ml_dtypes.float8_e3m4
DR = mybir.MatmulPerfMode.DoubleRow
ADD = mybir.AluOpType.add
MULT = mybir.AluOpType.mult
ISEQ = mybir.AluOpType.is_equal
ISGT = mybir.AluOpType.is_gt
EXPF = mybir.ActivationFunctionType.Exp
LNF = mybir.ActivationFunctionType.Ln
COPY = mybir.ActivationFunctionType.Copy

# accs column map
C_EVHS = 0            # 8 cols: sum of ev*headsum per lh group
C_NEV = 8             # 8 cols: event counts per lh group
C_EVLD = 16           # 1 col: sum of ev*log(denom)
C_PICK = 17           # 11 cols: sum of picked head-sum logits per ce group
C_LSE = 28            # 11 cols (partitions 0..23): sum of logsumexp per group
C_TOT = 48


def _lh_weights():
    """lhsT for the Cox head-sum: [128, 32] fp8e3.
    contract p = 16*ro + 4*ml + k; out o = 4*ro + k (heads split over two
    accumulating matmuls)."""
    w = np.zeros((P, 32), np.float32)
    for ro in range(8):
        for ml in range(4):
            for k in range(K):
                w[16 * ro + 4 * ml + k, 4 * ro + k] = 1.0
    return w.astype(E3NP)


def _lg_weights():
    """lhsT for the CE head-sum: [120, 32] fp8e3.
    contract p = 20*ro6 + 5*ml + c5; out o = 5*ro6 + c5 (o=30,31 stay zero;
    heads split over two accumulating matmuls)."""
    w = np.zeros((120, 32), np.float32)
    for ro6 in range(6):
        for ml in range(4):
            for c5 in range(NUM_CLS):
                w[20 * ro6 + 5 * ml + c5, 5 * ro6 + c5] = 1.0
    return w.astype(E3NP)


def _csum_weights():
    """Class-sum lhsT [128, 24] bf16: out u = 6*j2+ro6 sums classes of row
    (q = 32*j2 + 5*ro6 + c5)."""
    w = np.zeros((P, 24), np.float32)
    for j2 in range(4):
        for ro6 in range(6):
            for c5 in range(NUM_CLS):
                w[32 * j2 + 5 * ro6 + c5, 6 * j2 + ro6] = 1.0
    return w.astype(ml_dtypes.bfloat16)


def _slot_of_q():
    q = np.arange(P)
    return 8 * (q // 32) + (q % 32) // 4  # slot s(q); cause k(q) = q % 4


def _tri_cause():
    """triSC[q', q] = 1 if slot(q')<slot(q) and k(q')==k(q)  (fp32)."""
    s = _slot_of_q()
    k = np.arange(P) % 4
    return ((s[:, None] > s[None, :]) & (k[:, None] == k[None, :])).astype(np.float32).T


def _cause_match():
    k = np.arange(P) % 4
    return (k[:, None] == k[None, :]).astype(np.float32)


def build_nc(reps=1):
    nc = bacc.Bacc("TRN2", debug=False, num_devices=N_CORES)
    lh8 = nc.dram_tensor("lh8", [P, LH_MM * 1024], FP8E3, kind="ExternalInput")
    ev8 = nc.dram_tensor("ev8", [P, LH_G * 512], FP8E3, kind="ExternalInput")
    lg8 = nc.dram_tensor("lg8", [120, LG_MM * 1024], FP8E3, kind="ExternalInput")
    lb8 = nc.dram_tensor("lb8", [P, LG_G * 512], FP8E3, kind="ExternalInput")
    trid = nc.dram_tensor("trid", [P, P], F32, kind="ExternalInput")
    cmd_ = nc.dram_tensor("cmd", [P, P], F32, kind="ExternalInput")
    seld = nc.dram_tensor("seld", [N_CORES, 1], F32, kind="ExternalInput")
    lhwd = nc.dram_tensor("lhwd", [P, 32], FP8E3, kind="ExternalInput")
    lgwd = nc.dram_tensor("lgwd", [120, 32], FP8E3, kind="ExternalInput")
    cswd = nc.dram_tensor("cswd", [P, 24], BF16, kind="ExternalInput")
    accs = nc.dram_tensor("accs", [P, C_TOT], F32, kind="ExternalOutput")
    # collectives cannot touch I/O tensors -> internal DRAM bounce buffers
    cc_in = nc.dram_tensor("cc_in", [1, P], F32)
    cc_out = nc.dram_tensor("cc_out", [N_CORES, P], F32)

    with tile.TileContext(nc) as tc, ExitStack() as ctx:
        persist = ctx.enter_context(tc.tile_pool(name="persist", bufs=1))
        lhp = ctx.enter_context(tc.tile_pool(name="lhp", bufs=3))
        lgp = ctx.enter_context(tc.tile_pool(name="lgp", bufs=3))
        work = ctx.enter_context(tc.tile_pool(name="work", bufs=2))
        psum = ctx.enter_context(tc.tile_pool(name="psum", bufs=1, space="PSUM"))
        psum2 = ctx.enter_context(tc.tile_pool(name="psum2", bufs=2, space="PSUM"))

        # ---- persistent state / constants (setup, outside the rep body) ----
        trit = persist.tile([P, P], F32)
        cmt = persist.tile([P, P], F32)
        selt = persist.tile([N_CORES, 1], F32)
        lhwt = persist.tile([P, 32], FP8E3)
        lgwt = persist.tile([120, 32], FP8E3)
        cswt = persist.tile([P, 24], BF16)
        zeros4k = persist.tile([P, LH_G * 512], F32)
        evs = persist.tile([P, LH_G * 512], FP8E3)
        lbadj = persist.tile([P, LG_G * 512], FP8E3)
        w4k = persist.tile([P, LH_G * 512], F32)
        cum = persist.tile([P, LH_G * 512], F32)
        logd = persist.tile([P, LH_G * 512], BF16)
        scrl = persist.tile([P, LH_G * 512], BF16)
        ccs = persist.tile([1, P], F32)
        cc8 = persist.tile([N_CORES, P], F32)
        pa_sb = persist.tile([P, 1], F32)
        combf = persist.tile([P, 1], F32)
        acc_out = persist.tile([P, C_TOT], F32)

        nc.sync.dma_start(trit[:], trid[:, :])
        nc.sync.dma_start(cmt[:], cmd_[:, :])
        nc.sync.dma_start(selt[:], seld[:, :])
        nc.sync.dma_start(lhwt[:], lhwd[:, :])
        nc.sync.dma_start(lgwt[:], lgwd[:, :])
        nc.sync.dma_start(cswt[:], cswd[:, :])
        nc.vector.memset(zeros4k[:], 0.0)
        nc.vector.memset(acc_out[:], 0.0)

        def emit_ce_group(gi):
            t2 = lgp.tile([120, 4096], FP8E3, tag="lgt")
            nc.sync.dma_start(t2[:], lg8[:, gi * 4096:(gi + 1) * 4096])
            pce = psum2.tile([P, 512], F32, tag="pce")
            for j2 in range(4):
                for h in range(2):
                    rhs = t2[:, j2 * 1024 + h * 512:j2 * 1024 + (h + 1) * 512]
                    nc.tensor.matmul(pce[32 * j2:32 * j2 + 32, :], lgwt[:], rhs,
                                     start=(h == 0), stop=(h == 1),
                                     tile_position=(0, 32 * j2))
            eb = work.tile([P, 512], BF16, tag="eb")
            nc.scalar.activation(eb[:], pce[:], EXPF, bias=0.0, scale=1.0 / M)
            seP = psum2.tile([24, 512], F32, tag="seP")
            nc.tensor.matmul(seP[:], cswt[:], eb[:], start=True, stop=True)
            lseb = work.tile([24, 512], BF16, tag="lseb")
            nc.scalar.activation(lseb[:], seP[:], LNF, bias=0.0, scale=1.0,
                                 accum_out=acc_out[0:24, C_LSE + gi:C_LSE + gi + 1])
            scrp = work.tile([P, 512], BF16, tag="scrp")
            nc.vector.scalar_tensor_tensor(
                scrp[:], lbadj[:, gi * 512:(gi + 1) * 512], 0.0, pce[:],
                ISEQ, MULT,
                accum_out=acc_out[:, C_PICK + gi:C_PICK + gi + 1])

        def emit_rep():
            # ---------- Cox stream ----------
            nc.sync.dma_start(evs[:], ev8[:, :])
            for g in range(LH_G):
                t = lhp.tile([P, 4096], FP8E3, tag="lht")
                nc.sync.dma_start(t[:], lh8[:, g * 4096:(g + 1) * 4096])
                pg = psum2.tile([P, 512], F32, tag="pg")
                for j in range(4):
                    for h in range(2):
                        rhs = t[:, j * 1024 + h * 512:j * 1024 + (h + 1) * 512]
                        nc.tensor.matmul(pg[32 * j:32 * j + 32, :], lhwt[:], rhs,
                                         start=(h == 0), stop=(h == 1),
                                         tile_position=(0, 32 * j))
                nc.scalar.activation(w4k[:, g * 512:(g + 1) * 512], pg[:],
                                     EXPF, bias=0.0, scale=1.0 / M)
                evg = evs[:, g * 512:(g + 1) * 512]
                scr = work.tile([P, 512], F32, tag="scr")
                nc.vector.scalar_tensor_tensor(
                    scr[:], evg, 0.0, pg[:], ISGT, MULT,
                    accum_out=acc_out[:, C_EVHS + g:C_EVHS + g + 1])
                scr2 = work.tile([P, 512], BF16, tag="scr2")
                nc.vector.scalar_tensor_tensor(
                    scr2[:], evg, 0.0, evg, ISGT, MULT,
                    accum_out=acc_out[:, C_NEV + g:C_NEV + g + 1])

            # ---------- risk-set denominators ----------
            nc.vector.tensor_tensor_scan(
                cum[:], zeros4k[:], w4k[:], 0.0, op0=ADD, op1=ADD)
            totals = cum[:, LH_G * 512 - 1:LH_G * 512]
            small = psum.tile([P, 512], F32, tag="small")
            paP = small[:, 0:1]
            ccP = small[0:1, 128:256]
            pbP = small[:, 384:385]
            nc.tensor.matmul(paP, trit[:], totals, start=True, stop=True,
                             skip_group_check=True)
            nc.tensor.matmul(ccP, totals, cmt[:], start=True, stop=True,
                             skip_group_check=True)
            nc.scalar.copy(ccs[:], ccP)
            nc.scalar.activation(pa_sb[:], paP, COPY, bias=EPS)
            nc.gpsimd.dma_start(cc_in[0:1, :], ccs[:])
            nc.gpsimd.collective_compute(
                "AllGather", mybir.AluOpType.bypass,
                replica_groups=[list(range(N_CORES))],
                ins=[cc_in[:, :]], outs=[cc_out[:, :]],
            )
            nc.gpsimd.dma_start(cc8[:], cc_out[:, :])

            # ---------- CE stream part A (overlaps the collective) ----------
            nc.sync.dma_start(lbadj[:], lb8[:, :])
            for gi in range(6):
                emit_ce_group(gi)

            # ---------- Cox tail ----------
            nc.tensor.matmul(pbP, cc8[:], selt[:], start=True, stop=True,
                             skip_group_check=True)
            nc.vector.tensor_add(combf[:], pa_sb[:], pbP)
            nc.scalar.activation(logd[:], cum[:], LNF, bias=combf[:], scale=1.0)
            nc.vector.scalar_tensor_tensor(
                scrl[:], evs[:], 0.0, logd[:], ISGT, MULT,
                accum_out=acc_out[:, C_EVLD:C_EVLD + 1])

            # ---------- CE stream part B ----------
            for gi in range(6, LG_G):
                emit_ce_group(gi)

            nc.gpsimd.dma_start(accs[:, :], acc_out[:])

        for _rep in range(reps):
            emit_rep()

    nc.finalize()
    return nc


def prep_inputs(log_h, logits, durations, event_type, labels):
    """Host-side shard/permute/quantize.  Returns per-core in_maps + combine
    metadata."""
    n = log_h.shape[0]
    assert n == N

    order = np.argsort(-durations, kind="stable")
    lh_s = log_h[order]                     # (N, M, K) f32
    ev_s = event_type[order]                # (N,) int

    lhw = _lh_weights()
    lgw = _lg_weights()
    csw = _csum_weights()
    tri = _tri_cause()
    cm = _cause_match()

    padce8 = float(np.float32(np.float32(PADCE).astype(E3NP)))

    in_maps = []
    for c in range(N_CORES):
        lo = c * RC
        # --- Cox tensors ---
        lh_c = np.full((LH_ROWS, M, K), -15.0, np.float32)
        lh_c[:RC] = lh_s[lo:lo + RC]
        # row R = s*4096 + g*512 + c2 with slot s = 8j + ro -> row-major
        # nesting [j, ro, g, c2]; heads m = 4h + ml
        A = lh_c.reshape(4, 8, LH_G, 512, 2, 4, K)        # [j, ro, g, c2, h, ml, k]
        # partition p = 16ro + 4ml + k; column = ((g*4 + j)*2 + h)*512 + c2
        lh8 = np.ascontiguousarray(
            A.transpose(1, 5, 6, 2, 0, 4, 3).reshape(128, LH_MM * 1024))
        ev_c = np.zeros(LH_ROWS, np.int32)
        ev_c[:RC] = ev_s[lo:lo + RC]
        E = ev_c.reshape(4, 2, 4, LH_G, 512)              # [j, t, a, g, c2]
        # evs01[q = 16j+4a+k... q = 32*jq + 4*ro? q = 4*s + k with s = 8j+ro:
        # q = 32j + 4*ro + k = 32j + 16t + 4a + k
        # q = 32j + 16t + 4a + k -> row-major [j, t, a, k]
        ev01 = (E[:, :, :, None, :, :] ==
                (np.arange(1, K + 1)[None, None, None, :, None, None])
                ).astype(np.float32)                      # [j, t, a, k, g, c2]
        ev8 = np.ascontiguousarray(ev01.reshape(128, LH_G * 512))

        # --- CE tensors ---
        lg_c = np.full((LG_ROWS, M, NUM_CLS), PADCE, np.float32)
        lg_c[:RC] = logits[lo:lo + RC]
        lb_c = np.full(LG_ROWS, -9, np.int32)
        lb_c[:RC] = labels[lo:lo + RC]
        # row = (((gi*512 + c2)*4 + j2)*6 + ro6); heads m = 4h + ml
        B = lg_c.reshape(LG_G, 512, 4, 6, 2, 4, NUM_CLS)  # [gi, c2, j2, ro6, h, ml, c5]
        # partition p = 20ro6 + 5ml + c5; column = ((gi*4 + j2)*2 + h)*512 + c2
        lg8 = np.ascontiguousarray(
            B.transpose(3, 5, 6, 0, 2, 4, 1).reshape(120, LG_MM * 1024))
        LB = lb_c.reshape(LG_G, 512, 4, 2, 3)             # [gi, c2, j2, t, rr]
        # lbadj[q = 32j2 + 5*(3t+rr) + c5, gi*512 + c2] = label - c5
        lbq = np.full((LG_G, 512, 4, 8, 5), -9, np.int32)  # [gi, c2, j2, (ro6+2), c5]
        lbq[:, :, :, 0:6, :] = (LB.transpose(0, 1, 2, 3, 4).reshape(
            LG_G, 512, 4, 6)[:, :, :, :, None] - np.arange(5)[None, None, None, None, :])
        # wait: ro6 index order inside [t, rr] is ro6 = 3t + rr -> reshape(…, 6) maps
        # (t, rr) -> 3t + rr correctly since rr is innermost of (t, rr).
        lbq2 = np.full((LG_G, 512, 4, 32), -9, np.int32)
        lbq2[:, :, :, 0:30] = lbq[:, :, :, 0:6, :].reshape(LG_G, 512, 4, 30)
        lb8 = np.ascontiguousarray(
            lbq2.transpose(2, 3, 0, 1).reshape(128, LG_G * 512))

        sel = np.zeros((N_CORES, 1), np.float32)
        sel[:c] = 1.0

        in_maps.append({
            "lh8": lh8.astype(E3NP),
            "ev8": ev8.astype(E3NP),
            "lg8": lg8.astype(E3NP),
            "lb8": lb8.astype(np.float32).astype(E3NP),
            "trid": tri,
            "cmd": cm,
            "seld": sel,
            "lhwd": lhw,
            "lgwd": lgw,
            "cswd": csw,
        })

    n_pad_ce = N_CORES * (LG_ROWS - RC)
    pad_lse = n_pad_ce * float(np.log(NUM_CLS * np.exp(padce8)))
    return in_maps, pad_lse


def combine(results, pad_lse):
    """Host-side f64 combine of the per-core [128, 48] partials."""
    a = np.stack([r["accs"] for r in results]).astype(np.float64)  # [C, 128, 48]
    kq = np.arange(P) % 4
    s_eta = np.zeros(K)
    s_nev = np.zeros(K)
    s_logd = np.zeros(K)
    for k in range(K):
        sel = kq == k
        s_eta[k] = a[:, sel, C_EVHS:C_EVHS + LH_G].sum()
        s_nev[k] = a[:, sel, C_NEV:C_NEV + LH_G].sum()
        s_logd[k] = a[:, sel, C_EVLD].sum()
    s_pick = a[:, :, C_PICK:C_PICK + LG_G].sum()
    s_lse = a[:, 0:24, C_LSE:C_LSE + LG_G].sum() - pad_lse

    loss_c = -(s_eta / M - s_logd) / (s_nev + EPS)
    loss_surv = loss_c.sum()
    loss_cls = (s_lse - s_pick / M) / N
    return np.float32(ALPHA * loss_surv + (1.0 - ALPHA) * loss_cls)


_NC_CACHE = {}


def _get_nc(reps=1):
    if reps not in _NC_CACHE:
        _NC_CACHE[reps] = build_nc(reps)
    return _NC_CACHE[reps]


def kernel(log_h, logits, durations, event_type, labels):
    log_h = np.asarray(log_h, dtype=np.float32)
    logits = np.asarray(logits, dtype=np.float32)
    durations = np.asarray(durations, dtype=np.float32)
    event_type = np.asarray(event_type)
    labels = np.asarray(labels)
    nc = _get_nc(1)
    in_maps, pad_lse = prep_inputs(log_h, logits, durations, event_type, labels)
    try:
        res = run_bass_kernel_spmd(nc, in_maps, list(range(N_CORES)))
    except Exception as e:  # transient NRT errors after fresh compile
        if "UNRECOVERABLE" not in str(e) and "UNAVAILABLE" not in str(e):
            raise
        res = run_bass_kernel_spmd(nc, in_maps, list(range(N_CORES)))
    return np.array(combine(res.results, pad_lse), dtype=np.float32)


def _make_runner(nc, in_maps):
    """Steady-state runner: jitted shard_map with device-resident inputs."""
    import jax
    from jax.sharding import Mesh, PartitionSpec, NamedSharding
    from jax.experimental.shard_map import shard_map
    from concourse import bass2jax, mybir as mb

    bass2jax.install_neuronx_cc_hook()
    in_names, out_names, out_avals, zero_outs = [], [], [], []
    partition_name = nc.partition_id_tensor.name if nc.partition_id_tensor else None
    for alloc in nc.m.functions[0].allocations:
        if not isinstance(alloc, mb.MemoryLocationSet):
            continue
        name = alloc.memorylocations[0].name
        if alloc.kind == "ExternalInput":
            if name != partition_name:
                in_names.append(name)
        elif alloc.kind == "ExternalOutput":
            out_names.append(name)
            out_avals.append(jax.core.ShapedArray(
                tuple(alloc.tensor_shape), mb.dt.np(alloc.dtype)))
            zero_outs.append(np.zeros(alloc.tensor_shape, mb.dt.np(alloc.dtype)))
    n_params = len(in_names)
    all_in_names = list(in_names) + list(out_names)
    if partition_name is not None:
        all_in_names.append(partition_name)

    def _body(*args):
        operands = list(args)
        if partition_name is not None:
            operands.append(bass2jax.partition_id_tensor())
        outs = bass2jax._bass_exec_p.bind(
            *operands, out_avals=tuple(out_avals),
            in_names=tuple(all_in_names), out_names=tuple(out_names),
            lowering_input_output_aliases=(),
            sim_require_finite=True, sim_require_nnan=True, nc=nc)
        return tuple(outs)

    devices = jax.devices()[:N_CORES]
    mesh = Mesh(np.asarray(devices), ("core",))
    in_specs = (PartitionSpec("core"),) * (n_params + len(out_names))
    out_specs = (PartitionSpec("core"),) * len(out_names)
    sharded = jax.jit(
        shard_map(_body, mesh=mesh, in_specs=in_specs, out_specs=out_specs,
                  check_rep=False),
        donate_argnums=tuple(range(n_params, n_params + len(out_names))),
        keep_unused=True)
    sh = NamedSharding(mesh, PartitionSpec("core"))
    dev_in = [jax.device_put(
        np.concatenate([np.asarray(in_maps[c][nm]) for c in range(N_CORES)],
                       axis=0), sh)
        for nm in in_names]

    def call():
        zeros = [np.zeros((N_CORES * z.shape[0], *z.shape[1:]), z.dtype)
                 for z in zero_outs]
        outs = sharded(*dev_in, *zeros)
        jax.block_until_ready(outs)
        return outs

    return call


def measure_exec_ns(inputs, r_hi=101, samples=12):
    """Median-of-differences between reps=1 and reps=r_hi NEFFs, with
    device-resident inputs.  Robust to multi-ms axon dispatch jitter."""
    import time

    in_maps, _ = prep_inputs(np.asarray(inputs["log_h"], np.float32),
                             np.asarray(inputs["logits"], np.float32),
                             np.asarray(inputs["durations"], np.float32),
                             np.asarray(inputs["event_type"]),
                             np.asarray(inputs["labels"]))
    call_lo = _make_runner(_get_nc(1), in_maps)
    call_lo()
    call_hi = _make_runner(_get_nc(r_hi), in_maps)
    call_hi()

    lo_ts, hi_ts = [], []
    for _ in range(samples):
        t0 = time.perf_counter(); call_lo(); lo_ts.append(time.perf_counter() - t0)
        t0 = time.perf_counter(); call_hi(); hi_ts.append(time.perf_counter() - t0)
    lo = np.percentile(lo_ts, 25)
    hi = np.percentile(hi_ts, 25)
    per = (hi - lo) / (r_hi - 1)
    print(f"  [steady-state wall: p25 reps=1 {lo*1e3:.1f} ms, "
          f"reps={r_hi} {hi*1e3:.1f} ms -> {per*1e6:.1f} us/iter]")
    return max(per, 0.0) * 1e9
